# revision 19
# baseline (speedup 1.0000x reference)
"""Trainium2 Bass kernel for nn_Attention_37855841747487.

Dense transformer attention block: QKV projection, per-head L2-norm with
gamma * sqrt(d), xPos rotary embedding, GQA softmax attention (16 q heads,
4 kv heads), output projection with residual + bias.

Sharding: 8 cores = 2 batches x 4 query-row slices of 512. Each core
computes K/V for its full batch (duplicated across the 4 cores of that
batch) and attention + output projection for its 512 query rows. No
collectives.

v5 design (vs the bf16 v1 at ~345us; HW rel l2 err 1.689e-3):
- All projections (QKV, O) and attention@V run in fp8-e4m3 with DoubleRow
  perf mode (256-deep contraction per instruction: 2x bf16 PE array
  throughput AND half the instruction count -- the PE sequencer's ~174ns
  per-matmul decode is a first-order cost at this problem size). Weights
  are pre-scaled by 64 on the host so their 0.02-sigma values leave the
  e4m3 subnormal range; the scale cancels in l2norm for Q/K and is divided
  back out in the V-evacuation / Y-evacuation.
- Scores stay bf16 (contraction is d=64, so fp8 DoubleRow cannot shorten
  the instruction; the 16.8M-element score output bounds PE array time at
  one psum column per cycle regardless of dtype).
- Softmax exp is computed per 256-key block straight from PSUM into
  fp8-e5m2 probabilities in ONE op per block (constant scale/bias), split
  across ScalarE (exact Exp activation) and VectorE (Schraudolph fast-exp:
  a fused multiply-add whose uint8 result bit-pattern IS the e5m2 exp;
  bits stay in [19, 101] for this data's score range so no saturation path
  is exercised, and softmax's shared denominator cancels the correlated
  piecewise-linear error). GpSimd cannot read PSUM on TRN2, so it gets
  the SBUF-side work (norm multiplies, division partition_broadcast).
- attention@V keeps the transposed S^T[keys, q] orientation (out rows =
  [d | denominator-from-ones-column | zero-pad to 128: DoubleRow requires
  stationary columns in {32, 64, 128}]): one DoubleRow instruction per
  (head, key-pair-block), and the result lands directly in the ao^T
  layout the output projection consumes -- no transposes in phase C.
- k^T / q^T are built by SBUF->SBUF xbar DMA-transposes (3D out AP packs
  two 64-row heads per 128 partitions) instead of PE transposes; the
  norm/rope epilogues run as chunked whole-tensor ops interleaved with
  the projection loop so the vector chain overlaps PE work.
- The output projection takes ao^T as the stationary operand and streams
  natural Wo, producing natural-layout Y directly: no fp32 transposes
  anywhere. bo is folded into the residual on the host.
Measured (numpy model of the full quantization pipeline): rel l2 err
~1.7e-3 vs fp64 reference.
"""

import sys

sys.path.insert(0, "/opt/trn_rl_repo")

import math
import os

import numpy as np

# phase-bisection knob for sim diagnostics only ('A', 'AB', or 'ABC')
PHASES = os.environ.get('K_PHASES', 'ABC')

B, N, DIM = 2, 2048, 1024
H, KVH, D = 16, 4, 64
XPOS_SB = 4096
QS = N // 4  # query rows per core
NCORES = 8

# Schraudolph fast-exp to fp8-e5m2 bit patterns: P = exp(S/8) -> bits =
# 4*(log2 P + 15) = S * 4/(8 ln2) + 60.
SCH_E5_SLOPE = 0.5 / math.log(2.0)
SCH_E5_BIAS = 60.0
ACT_EXP_SCALE = 0.125

# exp engine rotation per (head, kt-pair) slot: 6 DVE, 10 Act.
# (GpSimd cannot read PSUM on TRN2, so it only gets SBUF-side work: the
# division's partition_broadcast and the norm multiplies.)
ROT = ['A', 'D', 'A', 'A', 'D', 'A', 'A', 'D',
       'A', 'A', 'D', 'A', 'A', 'D', 'A', 'D']

_CACHE = {}


# ---------------------------------------------------------------- host tables
def _make_tables(positions, scale_pow, gamma):
    """xPos rotary tables with rotate-half sign, gamma and rms folded in.

    Returns cosT, sinT of shape [n, Hg, 64]:
      roped(x) = x * cosT + swap_halves(x) * sinT   (applied pre-norm; the
    1/||x|| multiply happens separately on chip).
    """
    d = D
    half = np.arange(0, d, 2, dtype=np.float64)
    inv_freq = 1.0 / (10000.0 ** (half / d))
    t = positions.astype(np.float64)
    freqs = t[:, None] * inv_freq[None, :]
    freqs = np.concatenate([freqs, freqs], axis=-1)
    base_scale = (half + 0.4 * d) / (1.4 * d)
    power = (t - N // 2) / XPOS_SB
    scale = base_scale[None, :] ** power[:, None]
    scale = np.concatenate([scale, scale], axis=-1)
    scale = scale**scale_pow
    rms = np.sqrt(np.float64(D))
    cos = np.cos(freqs) * scale * rms
    sin = np.sin(freqs) * scale * rms
    sinA = np.concatenate([-sin[:, :32], sin[:, 32:]], axis=-1)
    gswap = np.concatenate([gamma[:, 32:], gamma[:, :32]], axis=-1)
    cosT = cos[:, None, :] * gamma[None, :, :]
    sinT = sinA[:, None, :] * gswap[None, :, :]
    return cosT.astype(np.float32), sinT.astype(np.float32)


# ---------------------------------------------------------------- bass kernel
def _build_nc(ht, htk, repeat=1):
    """Trace + compile the per-core program. ht/htk: table head dims (1 when
    gamma is all-ones and the head axis broadcasts, else H / KVH)."""
    import concourse.bacc as bacc
    import concourse.bass as bass
    import concourse.mybir as mybir
    import concourse.tile as tile
    from concourse.masks import make_identity

    f32 = mybir.dt.float32
    bf16 = mybir.dt.bfloat16
    e4 = mybir.dt.float8e4
    e5 = mybir.dt.float8e5
    u8 = mybir.dt.uint8
    AF = mybir.ActivationFunctionType
    AX = mybir.AxisListType
    OP = mybir.AluOpType
    DR = mybir.MatmulPerfMode.DoubleRow

    nc = bacc.Bacc("TRN2", target_bir_lowering=False, debug=False,
                   num_devices=NCORES, num_swdge_queues=4)

    xT8_d = nc.dram_tensor("xT8", [128, 8, N], e4, kind="ExternalInput")
    qxT8_d = nc.dram_tensor("qxT8", [128, 8, QS], e4, kind="ExternalInput")
    wq8_d = nc.dram_tensor("wq8", [128, 8, H * D], e4, kind="ExternalInput")
    wkv8_d = nc.dram_tensor("wkv8", [128, 8, 2 * KVH * D], e4, kind="ExternalInput")
    wo8_d = nc.dram_tensor("wo8", [128, 8, DIM], e4, kind="ExternalInput")
    qxr_d = nc.dram_tensor("qxr", [128, 4, DIM], f32, kind="ExternalInput")
    tqc_d = nc.dram_tensor("tqc", [128, QS // 128, ht, D], bf16, kind="ExternalInput")
    tqs_d = nc.dram_tensor("tqs", [128, QS // 128, ht, D], bf16, kind="ExternalInput")
    tkc_d = nc.dram_tensor("tkc", [128, N // 128, htk, D], bf16, kind="ExternalInput")
    tks_d = nc.dram_tensor("tks", [128, N // 128, htk, D], bf16, kind="ExternalInput")
    y_d = nc.dram_tensor("y", [QS, DIM], f32, kind="ExternalOutput")

    from contextlib import ExitStack

    with tile.TileContext(nc) as tc, ExitStack() as ctx:
        persist = ctx.enter_context(tc.tile_pool(name="persist", bufs=1))
        stage = ctx.enter_context(tc.tile_pool(name="stage", bufs=4))

        # ---- persistent SBUF tensors
        wq_sb = persist.tile([128, 8, H * D], e4)
        wkv_sb = persist.tile([128, 8, 2 * KVH * D], e4)
        qT_sb = persist.tile([128, 8, QS], bf16)      # roped, normalized q^T
        kT_sb = persist.tile([128, 2, N], bf16)       # roped, normalized k^T
        v_sb = persist.tile([128, 16, KVH * 128], e4)  # v | ones | zero-pad
        aoT_sb = persist.tile([128, 8, QS], e4)       # attention out^T, fp8
        tqc_sb = persist.tile([128, QS // 128, ht, D], bf16)
        tqs_sb = persist.tile([128, QS // 128, ht, D], bf16)
        tkc_sb = persist.tile([128, N // 128, htk, D], bf16)
        tks_sb = persist.tile([128, N // 128, htk, D], bf16)
        ident_bf = persist.tile([128, 128], bf16)
        make_identity(nc, ident_bf)
        v4 = v_sb.rearrange("p a (kv e) -> p a kv e", e=128)
        nc.vector.memset(v4[:, :, :, D : D + 1], 1.0)
        nc.vector.memset(v4[:, :, :, D + 1 : 128], 0.0)
        persist.seal()

        for _rep in range(repeat):
            # ---- A0: stream weights + x^T (pre-transposed, pre-cast on host)
            xp_ctx = tc.tile_pool(name="x_pool", bufs=1)
            x_pool = xp_ctx.__enter__()
            xT_sb = x_pool.tile([128, 8, N], e4)
            qxT_sb = x_pool.tile([128, 8, QS], e4)
            x_pool.seal()
            nc.sync.dma_start(out=wkv_sb, in_=wkv8_d[:, :, :])
            for c in range(4):
                nc.sync.dma_start(out=xT_sb[:, :, c * 512 : (c + 1) * 512],
                                  in_=xT8_d[:, :, c * 512 : (c + 1) * 512])
            nc.sync.dma_start(out=qxT_sb, in_=qxT8_d[:, :, :])
            nc.sync.dma_start(out=wq_sb, in_=wq8_d[:, :, :])
            nc.sync.dma_start(out=tkc_sb, in_=tkc_d[:, :, :, :])
            nc.sync.dma_start(out=tks_sb, in_=tks_d[:, :, :, :])
            nc.sync.dma_start(out=tqc_sb, in_=tqc_d[:, :, :, :])
            nc.sync.dma_start(out=tqs_sb, in_=tqs_d[:, :, :, :])

            # ---- A1: projections, then BATCHED norm/rope epilogues.
            # Per-tile work is only the PSUM evacuation (Act bf16 copy + V
            # fp8 copy); the l2-norm and rope run as a handful of
            # whole-tensor ops (the per-call version cost ~200 small ops
            # and their cross-engine sync dominated the phase). k^T / q^T
            # are produced by xbar DMA-transposes instead of PE transposes.
            ap_ctx = tc.tile_pool(name="apool", bufs=1)
            apool = ap_ctx.__enter__()
            pbk = apool.tile([128, 16, KVH, D], bf16)
            khat = apool.tile([128, 16, KVH, D], bf16)
            r1k = apool.tile([128, 16, KVH, D], bf16)
            pbq = apool.tile([128, 2, 4, H // 2, D], bf16)
            qhat = apool.tile([128, 2, 4, H // 2, D], bf16)
            r1q = apool.tile([128, 2, 4, H // 2, D], bf16)
            ssk_sb = apool.tile([128, 16, KVH], f32)
            ssq_sb = apool.tile([128, 2, 4, H // 2], f32)
            apool.seal()
            with tc.tile_pool(name="kv_ps", bufs=2, space="PSUM") as kv_ps, \
                 tc.tile_pool(name="q_ps", bufs=2, space="PSUM") as q_ps:
                def k_epilogue(half):
                    """rope + 1/||k|| + transpose for m-tiles [8*half, 8*half+8).
                    Emitted mid-loop so the DVE/Pool chain overlaps the
                    remaining projections on PE."""
                    lo, hi = half * 8, half * 8 + 8
                    nrmk = stage.tile([128, 8, KVH], f32, tag="nrmk")
                    nc.scalar.activation(nrmk, ssk_sb[:, lo:hi], AF.Sqrt)
                    rsk = stage.tile([128, 8, KVH], f32, tag="rsk")
                    nc.vector.reciprocal(rsk, nrmk)
                    if htk == 1:
                        ck = tkc_sb[:, lo:hi].broadcast_to([128, 8, KVH, D])
                        sk = tks_sb[:, lo:hi].broadcast_to([128, 8, KVH, D])
                    else:
                        ck, sk = tkc_sb[:, lo:hi], tks_sb[:, lo:hi]
                    pk, kh, r1 = pbk[:, lo:hi], khat[:, lo:hi], r1k[:, lo:hi]
                    nc.vector.tensor_tensor(out=r1, in0=pk, in1=ck, op=OP.mult)
                    nc.vector.tensor_tensor(out=kh[:, :, :, 0:32],
                                            in0=pk[:, :, :, 32:64],
                                            in1=sk[:, :, :, 0:32], op=OP.mult)
                    nc.vector.tensor_tensor(out=kh[:, :, :, 32:64],
                                            in0=pk[:, :, :, 0:32],
                                            in1=sk[:, :, :, 32:64], op=OP.mult)
                    nc.vector.tensor_tensor(out=kh, in0=kh, in1=r1, op=OP.add)
                    rskb = rsk.unsqueeze(3).broadcast_to([128, 8, KVH, D])
                    nc.vector.tensor_tensor(out=kh[:, 0:4], in0=kh[:, 0:4],
                                            in1=rskb[:, 0:4], op=OP.mult)
                    nc.gpsimd.tensor_tensor(out=kh[:, 4:8], in0=kh[:, 4:8],
                                            in1=rskb[:, 4:8], op=OP.mult)
                    for mt in range(lo, hi):
                        nc.sync.dma_start_transpose(
                            out=kT_sb[:, :, mt * 128 : (mt + 1) * 128],
                            in_=khat[:, mt])

                for g in range(8):
                    kvp = kv_ps.tile([128, 2, 2 * KVH * D], f32)
                    for i in range(2):
                        mt = g * 2 + i
                        for s in range(4):
                            nc.tensor.matmul(
                                kvp[:, i, :],
                                lhsT=xT_sb[:, 2 * s : 2 * s + 2,
                                           mt * 128 : (mt + 1) * 128],
                                rhs=wkv_sb[:, 2 * s : 2 * s + 2, :],
                                start=(s == 0), stop=(s == 3), perf_mode=DR)
                    kv8 = kvp.rearrange("p a (g2 d) -> p a g2 d", d=D)
                    nc.scalar.mul(v4[:, g * 2 : (g + 1) * 2, :, 0:D],
                                  kv8[:, :, KVH : 2 * KVH, :], 1.0 / 64.0)
                    nc.scalar.copy(out=pbk[:, g * 2 : (g + 1) * 2],
                                   in_=kv8[:, :, 0:KVH, :])
                    sqg = stage.tile([128, 2, KVH, D], f32, tag="sqg")
                    nc.scalar.activation(sqg, kv8[:, :, 0:KVH, :], AF.Square)
                    nc.vector.tensor_reduce(ssk_sb[:, g * 2 : (g + 1) * 2],
                                            sqg, axis=AX.X, op=OP.add)
                    if g == 3:
                        k_epilogue(0)
                k_epilogue(1)

                for nn in range(2):
                    for m in range(4):
                        qp = q_ps.tile([128, 512], f32)
                        for s in range(4):
                            nc.tensor.matmul(
                                qp,
                                lhsT=qxT_sb[:, 2 * s : 2 * s + 2,
                                            m * 128 : (m + 1) * 128],
                                rhs=wq_sb[:, 2 * s : 2 * s + 2,
                                          nn * 512 : (nn + 1) * 512],
                                start=(s == 0), stop=(s == 3), perf_mode=DR)
                        nc.scalar.copy(
                            out=pbq[:, nn, m],
                            in_=qp.rearrange("p (h d) -> p h d", d=D))
                        sqg = stage.tile([128, 1, H // 2, D], f32, tag="sqg")
                        qpv = qp.rearrange("p (o h d) -> p o h d", o=1, d=D)
                        nc.scalar.activation(sqg, qpv, AF.Square)
                        nc.vector.tensor_reduce(
                            ssq_sb[:, nn, m : m + 1], sqg,
                            axis=AX.X, op=OP.add)

                    # ---- Q epilogue for this 8-head half, right after its
                    # projections so it overlaps the next half's matmuls
                    pbn, qhn, r1n = pbq[:, nn], qhat[:, nn], r1q[:, nn]
                    nrmq = stage.tile([128, 4, H // 2], f32, tag="nrmq")
                    nc.scalar.activation(nrmq, ssq_sb[:, nn], AF.Sqrt)
                    rsq = stage.tile([128, 4, H // 2], f32, tag="rsq")
                    nc.vector.reciprocal(rsq, nrmq)
                    if ht == 1:
                        cqN = tqc_sb.broadcast_to([128, 4, H // 2, D])
                        sqN = tqs_sb.broadcast_to([128, 4, H // 2, D])
                    else:
                        cqN = tqc_sb[:, :, nn * 8 : (nn + 1) * 8, :]
                        sqN = tqs_sb[:, :, nn * 8 : (nn + 1) * 8, :]
                    nc.vector.tensor_tensor(out=r1n, in0=pbn, in1=cqN,
                                            op=OP.mult)
                    nc.vector.tensor_tensor(out=qhn[:, :, :, 0:32],
                                            in0=pbn[:, :, :, 32:64],
                                            in1=sqN[:, :, :, 0:32], op=OP.mult)
                    nc.vector.tensor_tensor(out=qhn[:, :, :, 32:64],
                                            in0=pbn[:, :, :, 0:32],
                                            in1=sqN[:, :, :, 32:64],
                                            op=OP.mult)
                    nc.vector.tensor_tensor(out=qhn, in0=qhn, in1=r1n,
                                            op=OP.add)
                    rsqb = rsq.unsqueeze(3).broadcast_to([128, 4, H // 2, D])
                    nc.vector.tensor_tensor(out=qhn[:, 0:2], in0=qhn[:, 0:2],
                                            in1=rsqb[:, 0:2], op=OP.mult)
                    nc.gpsimd.tensor_tensor(out=qhn[:, 2:4], in0=qhn[:, 2:4],
                                            in1=rsqb[:, 2:4], op=OP.mult)
                    for m in range(4):
                        nc.sync.dma_start_transpose(
                            out=qT_sb[:, nn * 4 : (nn + 1) * 4,
                                      m * 128 : (m + 1) * 128],
                            in_=qhat[:, nn, m])
            ap_ctx.__exit__(None, None, None)
            xp_ctx.__exit__(None, None, None)
            if 'B' not in PHASES:
                continue

            # ---- B: attention per head (wo weights + residual stream in
            # concurrently). exp: PSUM fp32 scores -> fp8-e5m2 probabilities
            # in one op per 256-key block, rotated across ScalarE/DVE/GpSimd.
            wo_ctx = tc.tile_pool(name="wo_pool", bufs=1)
            wo_pool = wo_ctx.__enter__()
            wo_sb = wo_pool.tile([128, 8, DIM], e4)
            qxr_sb = wo_pool.tile([128, 4, DIM], f32)
            wo_pool.seal()
            nc.sync.dma_start(out=wo_sb, in_=wo8_d[:, :, :])
            nc.sync.dma_start(out=qxr_sb, in_=qxr_d[:, :, :])
            with tc.tile_pool(name="sT_ps", bufs=3, space="PSUM") as sT_ps, \
                 tc.tile_pool(name="oT_ps", bufs=2, space="PSUM") as oT_ps, \
                 tc.tile_pool(name="pT_pool", bufs=4) as pT_pool, \
                 tc.tile_pool(name="small", bufs=2) as small:
                for h in range(H):
                    kvh = h % KVH
                    jq, qp_off = h // 2, 64 * (h % 2)
                    ktile, kp_off = kvh // 2, 64 * (kvh % 2)
                    oT = oT_ps.tile([128, 512], f32)

                    def do_av(pT8, s):
                        nc.tensor.matmul(
                            oT,
                            lhsT=v4[:, 2 * s : 2 * s + 2, kvh, :],
                            rhs=pT8,
                            start=(s == 0), stop=(s == 7), perf_mode=DR)

                    pending = None
                    for s in range(8):
                        sT = sT_ps.tile([128, 2, 512], f32)
                        for i in range(2):
                            kt = 2 * s + i
                            nc.tensor.matmul(
                                sT[:, i, :],
                                lhsT=kT_sb[kp_off : kp_off + 64, ktile,
                                           kt * 128 : (kt + 1) * 128],
                                rhs=qT_sb[qp_off : qp_off + 64, jq, :],
                                start=True, stop=True)
                        pT8 = pT_pool.tile([128, 2, 512], e5)
                        slot = ROT[(h * 8 + s) % 16]
                        if slot == 'A':
                            nc.scalar.activation(pT8, sT, AF.Exp,
                                                 scale=ACT_EXP_SCALE)
                        else:
                            nc.vector.tensor_scalar(
                                out=pT8.bitcast(u8), in0=sT,
                                scalar1=SCH_E5_SLOPE, scalar2=SCH_E5_BIAS,
                                op0=OP.mult, op1=OP.add)
                        if pending is not None:
                            do_av(*pending)
                        pending = (pT8, s)
                    do_av(*pending)
                    recip = small.tile([1, 512], f32, tag="recip")
                    nc.vector.reciprocal(recip, oT[D : D + 1, :])
                    rb = small.tile([D, 512], f32, tag="rb")
                    nc.gpsimd.partition_broadcast(rb, recip)
                    nc.vector.tensor_tensor(
                        out=aoT_sb[qp_off : qp_off + 64, jq, :],
                        in0=oT[0:D, :], in1=rb, op=OP.mult)

            # ---- C: output projection (natural orientation), 1/64 de-scale
            # + residual(+bias) add, store
            if 'C' not in PHASES:
                wo_ctx.__exit__(None, None, None)
                continue
            with tc.tile_pool(name="y_ps", bufs=2, space="PSUM") as y_ps, \
                 tc.tile_pool(name="ystage", bufs=2) as ystage:
                for qt in range(4):
                    yp = y_ps.tile([128, 2, 512], f32)
                    for ch in range(2):
                        for s in range(4):
                            nc.tensor.matmul(
                                yp[:, ch, :],
                                lhsT=aoT_sb[:, 2 * s : 2 * s + 2,
                                            qt * 128 : (qt + 1) * 128],
                                rhs=wo_sb[:, 2 * s : 2 * s + 2,
                                          ch * 512 : (ch + 1) * 512],
                                start=(s == 0), stop=(s == 3), perf_mode=DR)
                    ysc = ystage.tile([128, DIM], f32, tag="ysc")
                    nc.scalar.mul(ysc, yp.rearrange("p a b -> p (a b)"),
                                  1.0 / 64.0)
                    yn = ystage.tile([128, DIM], f32, tag="yn")
                    nc.vector.tensor_tensor(out=yn, in0=ysc,
                                            in1=qxr_sb[:, qt, :], op=OP.add)
                    nc.sync.dma_start(out=y_d[qt * 128 : (qt + 1) * 128, :],
                                      in_=yn)
            wo_ctx.__exit__(None, None, None)

    nc.compile()
    return nc


def _get_nc(ht, htk, repeat=1):
    key = (ht, htk, repeat, PHASES)
    if key not in _CACHE:
        _CACHE[key] = _build_nc(ht, htk, repeat)
    return _CACHE[key]


# ---------------------------------------------------------------- entry point
def make_in_maps(x, Wq, Wkv, q_gamma, k_gamma, Wo, bo):
    import ml_dtypes
    bf = ml_dtypes.bfloat16
    e4 = ml_dtypes.float8_e4m3
    x = np.ascontiguousarray(np.asarray(x, dtype=np.float32))
    Wq64 = (np.asarray(Wq, dtype=np.float32) * 64.0).astype(e4)
    Wkv64 = (np.asarray(Wkv, dtype=np.float32) * 64.0).astype(e4)
    Wo64 = (np.asarray(Wo, dtype=np.float32) * 64.0).astype(e4)
    bo = np.asarray(bo, dtype=np.float32)
    qg = np.asarray(q_gamma, dtype=np.float64).reshape(H, D)
    kg = np.asarray(k_gamma, dtype=np.float64).reshape(KVH, D)

    ht = 1 if np.all(qg == 1.0) else H
    htk = 1 if np.all(kg == 1.0) else KVH

    def ttiles(a):  # [n, h, d] bf16 -> [128, n//128, h, d]
        n, h, d = a.shape
        return np.ascontiguousarray(
            a.astype(bf).reshape(n // 128, 128, h, d).transpose(1, 0, 2, 3))

    def ktiles(w):  # [dim, cols] fp8 -> [128, dim//128, cols]
        dim, cols = w.shape
        return np.ascontiguousarray(
            w.reshape(dim // 128, 128, cols).transpose(1, 0, 2))

    pos = np.arange(N)
    tkc, tks = _make_tables(pos, -1.0, kg[:htk])
    tkc_t, tks_t = ttiles(tkc), ttiles(tks)
    wq_t, wkv_t, wo_t = ktiles(Wq64), ktiles(Wkv64), ktiles(Wo64)

    in_maps = []
    for c in range(NCORES):
        bi, qi = c // 4, c % 4
        xT8 = ktiles(np.ascontiguousarray(x[bi].T).astype(e4))  # [128, 8, N]
        qpos = pos[qi * QS : (qi + 1) * QS]
        tqc, tqs = _make_tables(qpos, +1.0, qg[:ht])
        qxr = x[bi, qi * QS : (qi + 1) * QS] + bo[None, :]
        qxr_t = np.ascontiguousarray(
            qxr.reshape(4, 128, DIM).transpose(1, 0, 2))
        in_maps.append({
            "xT8": xT8,
            "qxT8": np.ascontiguousarray(xT8[:, :, qi * QS : (qi + 1) * QS]),
            "wq8": wq_t, "wkv8": wkv_t, "wo8": wo_t,
            "qxr": qxr_t,
            "tqc": ttiles(tqc), "tqs": ttiles(tqs),
            "tkc": tkc_t, "tks": tks_t,
        })
    return in_maps, (ht, htk)


def kernel(x, Wq, Wkv, q_gamma, k_gamma, Wo, bo):
    from concourse import bass_utils

    in_maps, (ht, htk) = make_in_maps(x, Wq, Wkv, q_gamma, k_gamma, Wo, bo)
    nc = _get_nc(ht, htk)
    res = bass_utils.run_bass_kernel_spmd(nc, in_maps,
                                          core_ids=list(range(NCORES)))
    out = np.zeros((B, N, DIM), np.float32)
    for c in range(NCORES):
        bi, qi = c // 4, c % 4
        out[bi, qi * QS : (qi + 1) * QS] = res.results[c]["y"]
    return out


# revision 21
# speedup vs baseline: 2.6500x; 2.6500x over previous
"""Trainium2 Bass kernel for nn_Attention_37855841747487.

Dense transformer attention block: QKV projection, per-head L2-norm with
gamma * sqrt(d), xPos rotary embedding, GQA softmax attention (16 q heads,
4 kv heads), output projection with residual + bias.

Sharding: 8 cores = 2 batches x 4 query-row slices of 512. Each core
computes K/V for its full batch (duplicated across the 4 cores of that
batch) and attention + output projection for its 512 query rows. No
collectives.

v5 design (vs the bf16 v1 at ~345us; HW rel l2 err 1.689e-3):
- All projections (QKV, O) and attention@V run in fp8-e4m3 with DoubleRow
  perf mode (256-deep contraction per instruction: 2x bf16 PE array
  throughput AND half the instruction count -- the PE sequencer's ~174ns
  per-matmul decode is a first-order cost at this problem size). Weights
  are pre-scaled by 64 on the host so their 0.02-sigma values leave the
  e4m3 subnormal range; the scale cancels in l2norm for Q/K and is divided
  back out in the V-evacuation / Y-evacuation.
- Scores stay bf16 (contraction is d=64, so fp8 DoubleRow cannot shorten
  the instruction; the 16.8M-element score output bounds PE array time at
  one psum column per cycle regardless of dtype).
- Softmax exp is computed per 256-key block straight from PSUM into
  fp8-e5m2 probabilities in ONE op per block (constant scale/bias), split
  across ScalarE (exact Exp activation) and VectorE (Schraudolph fast-exp:
  a fused multiply-add whose uint8 result bit-pattern IS the e5m2 exp;
  bits stay in [19, 101] for this data's score range so no saturation path
  is exercised, and softmax's shared denominator cancels the correlated
  piecewise-linear error). GpSimd cannot read PSUM on TRN2, so it gets
  the SBUF-side work (norm multiplies, division partition_broadcast).
- attention@V keeps the transposed S^T[keys, q] orientation (out rows =
  [d | denominator-from-ones-column | zero-pad to 128: DoubleRow requires
  stationary columns in {32, 64, 128}]): one DoubleRow instruction per
  (head, key-pair-block), and the result lands directly in the ao^T
  layout the output projection consumes -- no transposes in phase C.
- k^T / q^T are built by SBUF->SBUF xbar DMA-transposes (3D out AP packs
  two 64-row heads per 128 partitions) instead of PE transposes; the
  norm/rope epilogues run as chunked whole-tensor ops interleaved with
  the projection loop so the vector chain overlaps PE work.
- The output projection takes ao^T as the stationary operand and streams
  natural Wo, producing natural-layout Y directly: no fp32 transposes
  anywhere. bo is folded into the residual on the host.
Measured (numpy model of the full quantization pipeline): rel l2 err
~1.7e-3 vs fp64 reference.
"""

import sys

sys.path.insert(0, "/opt/trn_rl_repo")

import math
import os

import numpy as np

# phase-bisection knob for sim diagnostics only ('A', 'AB', or 'ABC')
PHASES = os.environ.get('K_PHASES', 'ABC')

B, N, DIM = 2, 2048, 1024
H, KVH, D = 16, 4, 64
XPOS_SB = 4096
QS = N // 4  # query rows per core
NCORES = 8

# Schraudolph fast-exp to fp8-e5m2 bit patterns: P = exp(S/8) -> bits =
# 4*(log2 P + 15) = S * 4/(8 ln2) + 60.
SCH_E5_SLOPE = 0.5 / math.log(2.0)
SCH_E5_BIAS = 60.0
ACT_EXP_SCALE = 0.125

# exp engine rotation per (head, kt-pair) slot: 6 DVE, 10 Act.
# (GpSimd cannot read PSUM on TRN2, so it only gets SBUF-side work: the
# division's partition_broadcast and the norm multiplies.)
ROT = ['A', 'D', 'A', 'A', 'D', 'A', 'A', 'D',
       'A', 'A', 'D', 'A', 'A', 'D', 'A', 'D']

_CACHE = {}


# ---------------------------------------------------------------- host tables
def _make_tables(positions, scale_pow, gamma):
    """xPos rotary tables with rotate-half sign, gamma and rms folded in.

    Returns cosT, sinT of shape [n, Hg, 64]:
      roped(x) = x * cosT + swap_halves(x) * sinT   (applied pre-norm; the
    1/||x|| multiply happens separately on chip).
    """
    d = D
    half = np.arange(0, d, 2, dtype=np.float64)
    inv_freq = 1.0 / (10000.0 ** (half / d))
    t = positions.astype(np.float64)
    freqs = t[:, None] * inv_freq[None, :]
    freqs = np.concatenate([freqs, freqs], axis=-1)
    base_scale = (half + 0.4 * d) / (1.4 * d)
    power = (t - N // 2) / XPOS_SB
    scale = base_scale[None, :] ** power[:, None]
    scale = np.concatenate([scale, scale], axis=-1)
    scale = scale**scale_pow
    rms = np.sqrt(np.float64(D))
    cos = np.cos(freqs) * scale * rms
    sin = np.sin(freqs) * scale * rms
    sinA = np.concatenate([-sin[:, :32], sin[:, 32:]], axis=-1)
    gswap = np.concatenate([gamma[:, 32:], gamma[:, :32]], axis=-1)
    cosT = cos[:, None, :] * gamma[None, :, :]
    sinT = sinA[:, None, :] * gswap[None, :, :]
    return cosT.astype(np.float32), sinT.astype(np.float32)


# ---------------------------------------------------------------- bass kernel
def _build_nc(ht, htk, repeat=1):
    """Trace + compile the per-core program. ht/htk: table head dims (1 when
    gamma is all-ones and the head axis broadcasts, else H / KVH)."""
    import concourse.bacc as bacc
    import concourse.bass as bass
    import concourse.mybir as mybir
    import concourse.tile as tile
    from concourse.masks import make_identity

    f32 = mybir.dt.float32
    bf16 = mybir.dt.bfloat16
    e4 = mybir.dt.float8e4
    e5 = mybir.dt.float8e5
    u8 = mybir.dt.uint8
    AF = mybir.ActivationFunctionType
    AX = mybir.AxisListType
    OP = mybir.AluOpType
    DR = mybir.MatmulPerfMode.DoubleRow

    nc = bacc.Bacc("TRN2", target_bir_lowering=False, debug=False,
                   num_devices=NCORES, num_swdge_queues=4)

    xT8_d = nc.dram_tensor("xT8", [128, 8, N], e4, kind="ExternalInput")
    qxT8_d = nc.dram_tensor("qxT8", [128, 8, QS], e4, kind="ExternalInput")
    wq8_d = nc.dram_tensor("wq8", [128, 8, H * D], e4, kind="ExternalInput")
    wkv8_d = nc.dram_tensor("wkv8", [128, 8, 2 * KVH * D], e4, kind="ExternalInput")
    wo8_d = nc.dram_tensor("wo8", [128, 8, DIM], e4, kind="ExternalInput")
    qxr_d = nc.dram_tensor("qxr", [128, 4, DIM], f32, kind="ExternalInput")
    tqc_d = nc.dram_tensor("tqc", [128, QS // 128, ht, D], bf16, kind="ExternalInput")
    tqs_d = nc.dram_tensor("tqs", [128, QS // 128, ht, D], bf16, kind="ExternalInput")
    tkc_d = nc.dram_tensor("tkc", [128, N // 128, htk, D], bf16, kind="ExternalInput")
    tks_d = nc.dram_tensor("tks", [128, N // 128, htk, D], bf16, kind="ExternalInput")
    y_d = nc.dram_tensor("y", [QS, DIM], f32, kind="ExternalOutput")

    from contextlib import ExitStack

    with tile.TileContext(nc) as tc, ExitStack() as ctx:
        persist = ctx.enter_context(tc.tile_pool(name="persist", bufs=1))
        stage = ctx.enter_context(tc.tile_pool(name="stage", bufs=4))

        # ---- persistent SBUF tensors
        wq_sb = persist.tile([128, 8, H * D], e4)
        wkv_sb = persist.tile([128, 8, 2 * KVH * D], e4)
        qT_sb = persist.tile([128, 8, QS], bf16)      # roped, normalized q^T
        kT_sb = persist.tile([128, 2, N], bf16)       # roped, normalized k^T
        v_sb = persist.tile([128, 16, KVH * 128], e4)  # v | ones | zero-pad
        aoT_sb = persist.tile([128, 8, QS], e4)       # attention out^T, fp8
        tqc_sb = persist.tile([128, QS // 128, ht, D], bf16)
        tqs_sb = persist.tile([128, QS // 128, ht, D], bf16)
        tkc_sb = persist.tile([128, N // 128, htk, D], bf16)
        tks_sb = persist.tile([128, N // 128, htk, D], bf16)
        ident_bf = persist.tile([128, 128], bf16)
        make_identity(nc, ident_bf)
        v4 = v_sb.rearrange("p a (kv e) -> p a kv e", e=128)
        nc.vector.memset(v4[:, :, :, D : D + 1], 1.0)
        nc.vector.memset(v4[:, :, :, D + 1 : 128], 0.0)
        persist.seal()

        for _rep in range(repeat):
            # ---- A0: stream weights + x^T (pre-transposed, pre-cast on host)
            xp_ctx = tc.tile_pool(name="x_pool", bufs=1)
            x_pool = xp_ctx.__enter__()
            xT_sb = x_pool.tile([128, 8, N], e4)
            qxT_sb = x_pool.tile([128, 8, QS], e4)
            x_pool.seal()
            nc.sync.dma_start(out=wkv_sb, in_=wkv8_d[:, :, :])
            for c in range(4):
                nc.sync.dma_start(out=xT_sb[:, :, c * 512 : (c + 1) * 512],
                                  in_=xT8_d[:, :, c * 512 : (c + 1) * 512])
            nc.sync.dma_start(out=qxT_sb, in_=qxT8_d[:, :, :])
            nc.sync.dma_start(out=wq_sb, in_=wq8_d[:, :, :])
            nc.sync.dma_start(out=tkc_sb, in_=tkc_d[:, :, :, :])
            nc.sync.dma_start(out=tks_sb, in_=tks_d[:, :, :, :])
            nc.sync.dma_start(out=tqc_sb, in_=tqc_d[:, :, :, :])
            nc.sync.dma_start(out=tqs_sb, in_=tqs_d[:, :, :, :])

            # ---- A1: projections, then BATCHED norm/rope epilogues.
            # Per-tile work is only the PSUM evacuation (Act bf16 copy + V
            # fp8 copy); the l2-norm and rope run as a handful of
            # whole-tensor ops (the per-call version cost ~200 small ops
            # and their cross-engine sync dominated the phase). k^T / q^T
            # are produced by xbar DMA-transposes instead of PE transposes.
            ap_ctx = tc.tile_pool(name="apool", bufs=1)
            apool = ap_ctx.__enter__()
            pbk = apool.tile([128, 16, KVH, D], bf16)
            khat = apool.tile([128, 16, KVH, D], bf16)
            r1k = apool.tile([128, 16, KVH, D], bf16)
            pbq = apool.tile([128, 2, 4, H // 2, D], bf16)
            qhat = apool.tile([128, 2, 4, H // 2, D], bf16)
            r1q = apool.tile([128, 2, 4, H // 2, D], bf16)
            ssk_sb = apool.tile([128, 16, KVH], f32)
            ssq_sb = apool.tile([128, 2, 4, H // 2], f32)
            apool.seal()
            with tc.tile_pool(name="kv_ps", bufs=2, space="PSUM") as kv_ps, \
                 tc.tile_pool(name="q_ps", bufs=2, space="PSUM") as q_ps:
                def k_epilogue(half):
                    """rope + 1/||k|| + transpose for m-tiles [8*half, 8*half+8).
                    Emitted mid-loop so the DVE/Pool chain overlaps the
                    remaining projections on PE."""
                    lo, hi = half * 8, half * 8 + 8
                    nrmk = stage.tile([128, 8, KVH], f32, tag="nrmk")
                    nc.scalar.activation(nrmk, ssk_sb[:, lo:hi], AF.Sqrt)
                    rsk = stage.tile([128, 8, KVH], f32, tag="rsk")
                    nc.vector.reciprocal(rsk, nrmk)
                    if htk == 1:
                        ck = tkc_sb[:, lo:hi].broadcast_to([128, 8, KVH, D])
                        sk = tks_sb[:, lo:hi].broadcast_to([128, 8, KVH, D])
                    else:
                        ck, sk = tkc_sb[:, lo:hi], tks_sb[:, lo:hi]
                    pk, kh, r1 = pbk[:, lo:hi], khat[:, lo:hi], r1k[:, lo:hi]
                    nc.vector.tensor_tensor(out=r1, in0=pk, in1=ck, op=OP.mult)
                    nc.vector.tensor_tensor(out=kh[:, :, :, 0:32],
                                            in0=pk[:, :, :, 32:64],
                                            in1=sk[:, :, :, 0:32], op=OP.mult)
                    nc.vector.tensor_tensor(out=kh[:, :, :, 32:64],
                                            in0=pk[:, :, :, 0:32],
                                            in1=sk[:, :, :, 32:64], op=OP.mult)
                    nc.vector.tensor_tensor(out=kh, in0=kh, in1=r1, op=OP.add)
                    rskb = rsk.unsqueeze(3).broadcast_to([128, 8, KVH, D])
                    nc.vector.tensor_tensor(out=kh[:, 0:4], in0=kh[:, 0:4],
                                            in1=rskb[:, 0:4], op=OP.mult)
                    nc.gpsimd.tensor_tensor(out=kh[:, 4:8], in0=kh[:, 4:8],
                                            in1=rskb[:, 4:8], op=OP.mult)
                    for mt in range(lo, hi):
                        nc.sync.dma_start_transpose(
                            out=kT_sb[:, :, mt * 128 : (mt + 1) * 128],
                            in_=khat[:, mt])

                for g in range(8):
                    kvp = kv_ps.tile([128, 2, 2 * KVH * D], f32)
                    for i in range(2):
                        mt = g * 2 + i
                        for s in range(4):
                            nc.tensor.matmul(
                                kvp[:, i, :],
                                lhsT=xT_sb[:, 2 * s : 2 * s + 2,
                                           mt * 128 : (mt + 1) * 128],
                                rhs=wkv_sb[:, 2 * s : 2 * s + 2, :],
                                start=(s == 0), stop=(s == 3), perf_mode=DR)
                    kv8 = kvp.rearrange("p a (g2 d) -> p a g2 d", d=D)
                    nc.scalar.mul(v4[:, g * 2 : (g + 1) * 2, :, 0:D],
                                  kv8[:, :, KVH : 2 * KVH, :], 1.0 / 64.0)
                    nc.scalar.copy(out=pbk[:, g * 2 : (g + 1) * 2],
                                   in_=kv8[:, :, 0:KVH, :])
                    sqg = stage.tile([128, 2, KVH, D], f32, tag="sqg")
                    nc.scalar.activation(sqg, kv8[:, :, 0:KVH, :], AF.Square)
                    nc.vector.tensor_reduce(ssk_sb[:, g * 2 : (g + 1) * 2],
                                            sqg, axis=AX.X, op=OP.add)
                    if g == 3:
                        k_epilogue(0)
                k_epilogue(1)

                for nn in range(2):
                    for m in range(4):
                        qp = q_ps.tile([128, 512], f32)
                        for s in range(4):
                            nc.tensor.matmul(
                                qp,
                                lhsT=qxT_sb[:, 2 * s : 2 * s + 2,
                                            m * 128 : (m + 1) * 128],
                                rhs=wq_sb[:, 2 * s : 2 * s + 2,
                                          nn * 512 : (nn + 1) * 512],
                                start=(s == 0), stop=(s == 3), perf_mode=DR)
                        nc.scalar.copy(
                            out=pbq[:, nn, m],
                            in_=qp.rearrange("p (h d) -> p h d", d=D))
                        sqg = stage.tile([128, 1, H // 2, D], f32, tag="sqg")
                        qpv = qp.rearrange("p (o h d) -> p o h d", o=1, d=D)
                        nc.scalar.activation(sqg, qpv, AF.Square)
                        nc.vector.tensor_reduce(
                            ssq_sb[:, nn, m : m + 1], sqg,
                            axis=AX.X, op=OP.add)

                    # ---- Q epilogue for this 8-head half, right after its
                    # projections so it overlaps the next half's matmuls
                    pbn, qhn, r1n = pbq[:, nn], qhat[:, nn], r1q[:, nn]
                    nrmq = stage.tile([128, 4, H // 2], f32, tag="nrmq")
                    nc.scalar.activation(nrmq, ssq_sb[:, nn], AF.Sqrt)
                    rsq = stage.tile([128, 4, H // 2], f32, tag="rsq")
                    nc.vector.reciprocal(rsq, nrmq)
                    if ht == 1:
                        cqN = tqc_sb.broadcast_to([128, 4, H // 2, D])
                        sqN = tqs_sb.broadcast_to([128, 4, H // 2, D])
                    else:
                        cqN = tqc_sb[:, :, nn * 8 : (nn + 1) * 8, :]
                        sqN = tqs_sb[:, :, nn * 8 : (nn + 1) * 8, :]
                    nc.vector.tensor_tensor(out=r1n, in0=pbn, in1=cqN,
                                            op=OP.mult)
                    nc.vector.tensor_tensor(out=qhn[:, :, :, 0:32],
                                            in0=pbn[:, :, :, 32:64],
                                            in1=sqN[:, :, :, 0:32], op=OP.mult)
                    nc.vector.tensor_tensor(out=qhn[:, :, :, 32:64],
                                            in0=pbn[:, :, :, 0:32],
                                            in1=sqN[:, :, :, 32:64],
                                            op=OP.mult)
                    nc.vector.tensor_tensor(out=qhn, in0=qhn, in1=r1n,
                                            op=OP.add)
                    rsqb = rsq.unsqueeze(3).broadcast_to([128, 4, H // 2, D])
                    nc.vector.tensor_tensor(out=qhn[:, 0:2], in0=qhn[:, 0:2],
                                            in1=rsqb[:, 0:2], op=OP.mult)
                    nc.gpsimd.tensor_tensor(out=qhn[:, 2:4], in0=qhn[:, 2:4],
                                            in1=rsqb[:, 2:4], op=OP.mult)
                    for m in range(4):
                        nc.sync.dma_start_transpose(
                            out=qT_sb[:, nn * 4 : (nn + 1) * 4,
                                      m * 128 : (m + 1) * 128],
                            in_=qhat[:, nn, m])
            ap_ctx.__exit__(None, None, None)
            xp_ctx.__exit__(None, None, None)
            if 'B' not in PHASES:
                continue

            # ---- B: attention per head (wo weights + residual stream in
            # concurrently). exp: PSUM fp32 scores -> fp8-e5m2 probabilities
            # in one op per 256-key block, rotated across ScalarE/DVE/GpSimd.
            wo_ctx = tc.tile_pool(name="wo_pool", bufs=1)
            wo_pool = wo_ctx.__enter__()
            wo_sb = wo_pool.tile([128, 8, DIM], e4)
            qxr_sb = wo_pool.tile([128, 4, DIM], f32)
            wo_pool.seal()
            nc.sync.dma_start(out=wo_sb, in_=wo8_d[:, :, :])
            nc.sync.dma_start(out=qxr_sb, in_=qxr_d[:, :, :])
            with tc.tile_pool(name="sT_ps", bufs=3, space="PSUM") as sT_ps, \
                 tc.tile_pool(name="oT_ps", bufs=2, space="PSUM") as oT_ps, \
                 tc.tile_pool(name="pT_pool", bufs=4) as pT_pool, \
                 tc.tile_pool(name="small", bufs=2) as small:
                for h in range(H):
                    kvh = h % KVH
                    jq, qp_off = h // 2, 64 * (h % 2)
                    ktile, kp_off = kvh // 2, 64 * (kvh % 2)
                    oT = oT_ps.tile([128, 512], f32)

                    def do_av(pT8, s):
                        nc.tensor.matmul(
                            oT,
                            lhsT=v4[:, 2 * s : 2 * s + 2, kvh, :],
                            rhs=pT8,
                            start=(s == 0), stop=(s == 7), perf_mode=DR)

                    pending = None
                    for s in range(8):
                        sT = sT_ps.tile([128, 2, 512], f32)
                        for i in range(2):
                            kt = 2 * s + i
                            nc.tensor.matmul(
                                sT[:, i, :],
                                lhsT=kT_sb[kp_off : kp_off + 64, ktile,
                                           kt * 128 : (kt + 1) * 128],
                                rhs=qT_sb[qp_off : qp_off + 64, jq, :],
                                start=True, stop=True)
                        pT8 = pT_pool.tile([128, 2, 512], e5)
                        slot = ROT[(h * 8 + s) % 16]
                        if slot == 'A':
                            nc.scalar.activation(pT8, sT, AF.Exp,
                                                 scale=ACT_EXP_SCALE)
                        else:
                            nc.vector.tensor_scalar(
                                out=pT8.bitcast(u8), in0=sT,
                                scalar1=SCH_E5_SLOPE, scalar2=SCH_E5_BIAS,
                                op0=OP.mult, op1=OP.add)
                        if pending is not None:
                            do_av(*pending)
                        pending = (pT8, s)
                    do_av(*pending)
                    recip = small.tile([1, 512], f32, tag="recip")
                    nc.vector.reciprocal(recip, oT[D : D + 1, :])
                    rb = small.tile([D, 512], f32, tag="rb")
                    nc.gpsimd.partition_broadcast(rb, recip)
                    nc.vector.tensor_tensor(
                        out=aoT_sb[qp_off : qp_off + 64, jq, :],
                        in0=oT[0:D, :], in1=rb, op=OP.mult)

            # ---- C: output projection (natural orientation), 1/64 de-scale
            # + residual(+bias) add, store
            if 'C' not in PHASES:
                wo_ctx.__exit__(None, None, None)
                continue
            with tc.tile_pool(name="y_ps", bufs=2, space="PSUM") as y_ps, \
                 tc.tile_pool(name="ystage", bufs=2) as ystage:
                for qt in range(4):
                    yp = y_ps.tile([128, 2, 512], f32)
                    for ch in range(2):
                        for s in range(4):
                            nc.tensor.matmul(
                                yp[:, ch, :],
                                lhsT=aoT_sb[:, 2 * s : 2 * s + 2,
                                            qt * 128 : (qt + 1) * 128],
                                rhs=wo_sb[:, 2 * s : 2 * s + 2,
                                          ch * 512 : (ch + 1) * 512],
                                start=(s == 0), stop=(s == 3), perf_mode=DR)
                    ysc = ystage.tile([128, DIM], f32, tag="ysc")
                    nc.scalar.mul(ysc, yp.rearrange("p a b -> p (a b)"),
                                  1.0 / 64.0)
                    yn = ystage.tile([128, DIM], f32, tag="yn")
                    nc.vector.tensor_tensor(out=yn, in0=ysc,
                                            in1=qxr_sb[:, qt, :], op=OP.add)
                    nc.sync.dma_start(out=y_d[qt * 128 : (qt + 1) * 128, :],
                                      in_=yn)
            wo_ctx.__exit__(None, None, None)

    nc.compile()
    return nc


def _get_nc(ht, htk, repeat=1):
    key = (ht, htk, repeat, PHASES)
    if key not in _CACHE:
        _CACHE[key] = _build_nc(ht, htk, repeat)
    return _CACHE[key]


# ---------------------------------------------------------------- entry point
def make_in_maps(x, Wq, Wkv, q_gamma, k_gamma, Wo, bo):
    import ml_dtypes
    bf = ml_dtypes.bfloat16
    e4 = ml_dtypes.float8_e4m3
    x = np.ascontiguousarray(np.asarray(x, dtype=np.float32))
    Wq64 = (np.asarray(Wq, dtype=np.float32) * 64.0).astype(e4)
    Wkv64 = (np.asarray(Wkv, dtype=np.float32) * 64.0).astype(e4)
    Wo64 = (np.asarray(Wo, dtype=np.float32) * 64.0).astype(e4)
    bo = np.asarray(bo, dtype=np.float32)
    qg = np.asarray(q_gamma, dtype=np.float64).reshape(H, D)
    kg = np.asarray(k_gamma, dtype=np.float64).reshape(KVH, D)

    ht = 1 if np.all(qg == 1.0) else H
    htk = 1 if np.all(kg == 1.0) else KVH

    def ttiles(a):  # [n, h, d] bf16 -> [128, n//128, h, d]
        n, h, d = a.shape
        return np.ascontiguousarray(
            a.astype(bf).reshape(n // 128, 128, h, d).transpose(1, 0, 2, 3))

    def ktiles(w):  # [dim, cols] fp8 -> [128, dim//128, cols]
        dim, cols = w.shape
        return np.ascontiguousarray(
            w.reshape(dim // 128, 128, cols).transpose(1, 0, 2))

    pos = np.arange(N)
    tkc, tks = _make_tables(pos, -1.0, kg[:htk])
    tkc_t, tks_t = ttiles(tkc), ttiles(tks)
    wq_t, wkv_t, wo_t = ktiles(Wq64), ktiles(Wkv64), ktiles(Wo64)

    in_maps = []
    for c in range(NCORES):
        bi, qi = c // 4, c % 4
        xT8 = ktiles(np.ascontiguousarray(x[bi].T).astype(e4))  # [128, 8, N]
        qpos = pos[qi * QS : (qi + 1) * QS]
        tqc, tqs = _make_tables(qpos, +1.0, qg[:ht])
        qxr = x[bi, qi * QS : (qi + 1) * QS] + bo[None, :]
        qxr_t = np.ascontiguousarray(
            qxr.reshape(4, 128, DIM).transpose(1, 0, 2))
        in_maps.append({
            "xT8": xT8,
            "qxT8": np.ascontiguousarray(xT8[:, :, qi * QS : (qi + 1) * QS]),
            "wq8": wq_t, "wkv8": wkv_t, "wo8": wo_t,
            "qxr": qxr_t,
            "tqc": ttiles(tqc), "tqs": ttiles(tqs),
            "tkc": tkc_t, "tks": tks_t,
        })
    return in_maps, (ht, htk)


def kernel(x, Wq, Wkv, q_gamma, k_gamma, Wo, bo):
    from concourse import bass_utils

    in_maps, (ht, htk) = make_in_maps(x, Wq, Wkv, q_gamma, k_gamma, Wo, bo)
    nc = _get_nc(ht, htk)
    res = bass_utils.run_bass_kernel_spmd(nc, in_maps,
                                          core_ids=list(range(NCORES)))
    out = np.zeros((B, N, DIM), np.float32)
    for c in range(NCORES):
        bi, qi = c // 4, c % 4
        out[bi, qi * QS : (qi + 1) * QS] = res.results[c]["y"]
    return out
